# revision 66
# baseline (speedup 1.0000x reference)
"""MultiHeadGraphAttention TRN2 kernel (v2 — ACT-paced pipeline).

Data-parallel over (batch, query-half): core c handles batch c//2, query rows
(c%2)*1024 .. +1024.  No collectives.  Matmuls in bf16 (fp32 PSUM), softmax
and LayerNorm in fp32.

The softmax exp is the hard engine floor: 16.8M score elements must pass
through ScalarE's ACTIVATE(Exp) at 1 elem/cycle/lane @1.2GHz = ~143us/core.
Everything is organized so ACT runs exp back-to-back and every other engine
hides underneath:

  - scores are computed TRANSPOSED (S^T[m, n]) in [128,1024] PSUM tiles
    (2 key tiles x 512 query cols), exp'd PSUM->SBUF bf16 by ACT, masked by
    DVE (multiply by 0/1 mask AFTER exp), then AV-matmul'd with an appended
    ones-column on V giving the softmax denominator for free.
  - score matmuls for an even/odd head pair are emitted adjacently with
    lhsT/rhs at base_partition 0/64, so they land on PE row-tiles T0/T8
    (64x128 mode) and stream CONCURRENTLY -> scores cost half the cycles.
  - the unit stream runs one unit of score-lookahead ahead of exp; AV and
    the Q/K/V/O projection matmuls trail as PE filler in ACT's slack.
  - LayerNorm: stats on DVE inline, but the Sqrt for all 8 row tiles is
    batched at the very end so ACT never thrashes activation table sets.
"""

import os
import sys

import numpy as np

try:
    import concourse  # noqa: F401
except ImportError:  # harness runs from a bare dir; the repo is a fixed path
    sys.path.insert(0, "/opt/trn_rl_repo")

import ml_dtypes

B, N, M, D, H, HD = 4, 2048, 2048, 512, 8, 64
NS = 1024          # query rows per core
NCORES = 8
LN_EPS = 1e-5
BF16 = ml_dtypes.bfloat16

_CACHE = {}

KC = D // 128      # 4 contraction chunks of 128
NT = NS // 128     # 8 query-row tiles
NCH = NS // 512    # 2 query-column chunks
MT = M // 128      # 16 key-position tiles
MCH = M // 512     # 4 key chunks of 512
NU = MT // 2       # 8 units per chunk (2 key tiles each)


def _build():
    import concourse.bass as bass  # noqa: F401
    import concourse.tile as tile
    from concourse import bacc, mybir
    from concourse.masks import make_identity

    f32 = mybir.dt.float32
    bf16 = mybir.dt.bfloat16
    Exp = mybir.ActivationFunctionType.Exp
    Sqrt = mybir.ActivationFunctionType.Sqrt
    sub = mybir.AluOpType.subtract
    mult = mybir.AluOpType.mult

    nc = bacc.Bacc(None, target_bir_lowering=False, debug=False)

    xqT_d = nc.dram_tensor("xqT", [D, NS], bf16, kind="ExternalInput")
    xkT_d = nc.dram_tensor("xkT", [D, M], bf16, kind="ExternalInput")
    xvT_d = nc.dram_tensor("xvT", [D, M], bf16, kind="ExternalInput")
    maskT_d = nc.dram_tensor("maskT", [M, NS], bf16, kind="ExternalInput")
    qres_d = nc.dram_tensor("qres", [NS, D], f32, kind="ExternalInput")
    wqT_d = nc.dram_tensor("wqT", [D, D], bf16, kind="ExternalInput")
    wkT_d = nc.dram_tensor("wkT", [D, D], bf16, kind="ExternalInput")
    wvT_d = nc.dram_tensor("wvT", [D, D], bf16, kind="ExternalInput")
    woT_d = nc.dram_tensor("woT", [D, D], bf16, kind="ExternalInput")
    gamma_d = nc.dram_tensor("gamma", [1, D], f32, kind="ExternalInput")
    beta_d = nc.dram_tensor("beta", [1, D], f32, kind="ExternalInput")
    out_d = nc.dram_tensor("out", [NS, D], f32, kind="ExternalOutput")

    with tile.TileContext(nc) as tc:
        with (
            tc.tile_pool(name="big", bufs=1) as big,
            tc.tile_pool(name="wpool", bufs=1) as wpool,
            tc.tile_pool(name="ptp", bufs=5) as ptp,
            tc.tile_pool(name="ypool", bufs=2) as ypool,
            tc.tile_pool(name="opool", bufs=2) as opool,
            tc.tile_pool(name="small", bufs=4) as small,
            tc.tile_pool(name="nsc", bufs=1) as nsc,
            tc.tile_pool(name="xvp", bufs=8) as xvp,
            tc.tile_pool(name="ps_s", bufs=2, space="PSUM") as ps_s,
            tc.tile_pool(name="ps_mm", bufs=2, space="PSUM") as ps_mm,
            tc.tile_pool(name="ps_o", bufs=2, space="PSUM") as ps_o,
        ):
            # ---- resident SBUF tensors -----------------------------------
            xqT = big.tile([128, KC, NS], bf16, tag="xqT")
            xkT = big.tile([128, KC, M], bf16, tag="xkT")
            maskT = big.tile([128, MT, NS], bf16, tag="maskT")
            qT = big.tile([128, KC, NS], bf16, tag="qT")
            kT = big.tile([128, KC, M], bf16, tag="kT")
            vS = big.tile([128, MT, H * (HD + 1)], bf16, tag="vS")
            oT0 = big.tile([128, KC, 512], bf16, tag="oT0")
            oT1 = big.tile([128, KC, 512], bf16, tag="oT1")
            ones64 = wpool.tile([1, 64], f32, tag="ones64")
            wq = wpool.tile([128, KC, D], bf16, tag="wq")
            wk = wpool.tile([128, KC, D], bf16, tag="wk")
            wv = wpool.tile([128, KC, D], bf16, tag="wv")
            wo = wpool.tile([128, KC, D], bf16, tag="wo")
            gamma_b = wpool.tile([128, D], f32, tag="gamma_b")
            beta_b = wpool.tile([128, D], f32, tag="beta_b")
            gamma_1 = wpool.tile([1, D], f32, tag="gamma_1")
            beta_1 = wpool.tile([1, D], f32, tag="beta_1")
            eps_t = wpool.tile([128, 1], f32, tag="eps")
            ident = wpool.tile([128, 128], f32, tag="ident")
            # LN staging (persist until the batched tail)
            vars8 = wpool.tile([128, NT], f32, tag="vars8")
            g_all = wpool.tile([128, NT, D], f32, tag="g_all")
            make_identity(nc, ident)

            # ---- input DMAs (critical-path slices first) -----------------
            xq_r = xqT_d[:].rearrange("(c p) n -> p c n", p=128)
            xk_r = xkT_d[:].rearrange("(c p) n -> p c n", p=128)
            mask_r = maskT_d[:].rearrange("(j p) n -> p j n", p=128)
            nc.sync.dma_start(out=wk, in_=wkT_d[:].rearrange("(c p) o -> p c o", p=128))
            nc.sync.dma_start(out=xkT[:, :, 0:512], in_=xk_r[:, :, 0:512])
            nc.sync.dma_start(out=wq, in_=wqT_d[:].rearrange("(c p) o -> p c o", p=128))
            nc.sync.dma_start(out=xqT[:, :, 0:512], in_=xq_r[:, :, 0:512])
            for mc in range(1, MCH):
                nc.sync.dma_start(
                    out=xkT[:, :, mc * 512 : (mc + 1) * 512],
                    in_=xk_r[:, :, mc * 512 : (mc + 1) * 512],
                )
            nc.sync.dma_start(out=xqT[:, :, 512:1024], in_=xq_r[:, :, 512:1024])
            nc.sync.dma_start(out=wv, in_=wvT_d[:].rearrange("(c p) o -> p c o", p=128))
            xvT_r = xvT_d[:].rearrange("(c p) (j m) -> p c j m", p=128, m=128)
            xv_tiles = {}
            for j in range(8):             # prefetch V slices ahead of masks
                xv_t = xvp.tile([128, KC, 128], bf16, tag="xv")
                nc.sync.dma_start(out=xv_t, in_=xvT_r[:, :, j, :])
                xv_tiles[j] = xv_t
            for j in range(MT):
                nc.sync.dma_start(out=maskT[:, j, :], in_=mask_r[:, j, :])
            nc.sync.dma_start(out=wo, in_=woT_d[:].rearrange("(c p) o -> p c o", p=128))
            nc.sync.dma_start(out=gamma_1, in_=gamma_d[:])
            nc.sync.dma_start(out=beta_1, in_=beta_d[:])
            nc.gpsimd.partition_broadcast(gamma_b, gamma_1, channels=128)
            nc.gpsimd.partition_broadcast(beta_b, beta_1, channels=128)
            nc.vector.memset(eps_t, LN_EPS)
            nc.vector.memset(ones64, 1.0)
            nc.vector.memset(
                vS[:].rearrange("p j (h x) -> p j h x", x=HD + 1)[:, :, :, HD : HD + 1],
                1.0,
            )

            # ---- projection emitters (PE filler work) --------------------
            def q_proj(t, ncc):
                ps = ps_mm.tile([128, 512], f32, tag="mm")
                for kc in range(KC):
                    nc.tensor.matmul(
                        ps,
                        lhsT=wq[:, kc, t * 128 : (t + 1) * 128],
                        rhs=xqT[:, kc, ncc * 512 : (ncc + 1) * 512],
                        start=(kc == 0),
                        stop=(kc == KC - 1),
                    )
                nc.vector.tensor_copy(
                    out=qT[:, t, ncc * 512 : (ncc + 1) * 512], in_=ps
                )

            def k_proj(t, mc):
                ps = ps_mm.tile([128, 512], f32, tag="mm")
                for kc in range(KC):
                    nc.tensor.matmul(
                        ps,
                        lhsT=wk[:, kc, t * 128 : (t + 1) * 128],
                        rhs=xkT[:, kc, mc * 512 : (mc + 1) * 512],
                        start=(kc == 0),
                        stop=(kc == KC - 1),
                    )
                nc.vector.tensor_copy(
                    out=kT[:, t, mc * 512 : (mc + 1) * 512], in_=ps
                )

            def v_fetch(j):
                xv_t = xvp.tile([128, KC, 128], bf16, tag="xv")
                nc.sync.dma_start(out=xv_t, in_=xvT_r[:, :, j, :])
                xv_tiles[j] = xv_t

            def v_proj(j):
                xv_t = xv_tiles.pop(j)
                ps = ps_mm.tile([128, 512], f32, tag="mm")
                for kc in range(KC):
                    nc.tensor.matmul(
                        ps,
                        lhsT=xv_t[:, kc, :],
                        rhs=wv[:, kc, :],
                        start=(kc == 0),
                        stop=(kc == KC - 1),
                    )
                # alternate evacuation engine: keep DVE under the ACT pace
                dst = vS[:, j, :].rearrange("p (h x) -> p h x", x=HD + 1)[:, :, 0:HD]
                src = ps[:].rearrange("p (h x) -> p h x", x=HD)
                if j % 2 == 0:
                    nc.vector.tensor_copy(out=dst, in_=src)
                else:
                    nc.scalar.copy(dst, src)

            # ---- out-projection stage 1 (per row tile, inline) -----------
            qres_r = qres_d[:].rearrange("(t p) d -> p t d", p=128)
            out_r = out_d[:].rearrange("(t p) d -> p t d", p=128)

            def out_tile_s1(nt):
                """Returns the out-projection + LN-stats work as stages."""
                box = {}

                def st_mm():
                    oTx = oT0 if nt < 4 else oT1
                    nb = (nt % 4) * 128
                    ps = ps_mm.tile([128, D], f32, tag="mm")
                    for a in range(KC):
                        nc.tensor.matmul(
                            ps,
                            lhsT=oTx[:, a, nb : nb + 128],
                            rhs=wo[:, a, :],
                            start=(a == 0),
                            stop=(a == KC - 1),
                        )
                    x_t = ypool.tile([128, D], f32, tag="qres")
                    nc.sync.dma_start(out=x_t, in_=qres_r[:, nt, :])
                    box["ps"], box["x"] = ps, x_t

                def st_stats():
                    ps, x_t = box["ps"], box["x"]
                    nc.vector.tensor_add(x_t, ps, x_t)
                    stats = small.tile([128, 6], f32, tag="stats")
                    nc.vector.bn_stats(out=stats, in_=x_t)
                    mv = small.tile([128, 2], f32, tag="mv")
                    nc.vector.bn_aggr(out=mv, in_=stats)
                    nc.vector.tensor_copy(out=vars8[:, nt : nt + 1], in_=mv[:, 1:2])
                    box["mv"] = mv

                def st_gam():
                    nc.vector.tensor_scalar(
                        out=g_all[:, nt, :], in0=box["x"], scalar1=box["mv"][:, 0:1],
                        scalar2=None, op0=sub,
                    )
                    nc.gpsimd.tensor_mul(g_all[:, nt, :], g_all[:, nt, :], gamma_b)

                return [st_mm, st_stats, st_gam]

            def ln_tail():
                rstd = wpool.tile([128, NT], f32, tag="rstd8")
                nc.scalar.activation(rstd, vars8, Sqrt, bias=eps_t)
                nc.vector.reciprocal(rstd, rstd)
                for nt in range(NT):
                    y_t = g_all[:, nt, :]  # in place: g dead after this
                    nc.vector.tensor_scalar(
                        out=y_t,
                        in0=y_t,
                        scalar1=rstd[:, nt : nt + 1],
                        scalar2=None,
                        op0=mult,
                    )
                    if nt % 2 == 0:        # split the serial tail chain
                        nc.gpsimd.tensor_add(y_t, y_t, beta_b)
                    else:
                        nc.vector.tensor_add(y_t, y_t, beta_b)
                    nc.sync.dma_start(out=out_r[:, nt, :], in_=y_t)

            # ---- softmax denominator reciprocal (PE transpose trick) -----
            # both heads of a chunk batched: d rows staged to SBUF, PE
            # transposes spread them over partitions so DVE reciprocal runs
            # lane-parallel, transposes back, broadcast, multiply.
            def normalize_pair(po_e, po_o, hp, ncc):
                """Returns the softmax-denominator normalize work as stages."""
                nsl = slice(ncc * 512, (ncc + 1) * 512)
                box = {}

                def st_d():
                    d_sb = nsc.tile([1, 1024], f32, tag="d_sb")
                    nc.vector.tensor_copy(out=d_sb[:, 0:512], in_=po_e[64:65, :])
                    nc.vector.tensor_copy(out=d_sb[:, 512:1024], in_=po_o[64:65, :])
                    box["d"] = d_sb

                def st_recip():
                    d_sb = box["d"]
                    scr = ps_mm.tile([128, 512], f32, tag="mm")
                    dT = scr[:, 0:8]
                    for c in range(2 * KC):
                        nc.tensor.transpose(
                            dT[:, c : c + 1],
                            d_sb[:, c * 128 : (c + 1) * 128],
                            ident[0:1, 0:1],
                        )
                    rT = small.tile([128, 8], f32, tag="rT")
                    nc.vector.reciprocal(rT, dT)
                    for half in range(2):
                        rr = scr[0:1, 0:512]
                        for c in range(KC):
                            nc.tensor.transpose(
                                rr[:, c * 128 : (c + 1) * 128],
                                rT[:, 4 * half + c : 4 * half + c + 1],
                                ident,
                            )
                        nc.vector.tensor_copy(
                            out=d_sb[:, half * 512 : (half + 1) * 512], in_=rr
                        )

                def mk_mul(par, po_t):
                    def st_mul():
                        # broadcast 1/d over 64 partitions via a PE rank-1
                        # outer product (no gpsimd in this chain)
                        rb_ps = ps_mm.tile([64, 512], f32, tag="mm")
                        nc.tensor.matmul(
                            rb_ps,
                            lhsT=ones64,
                            rhs=box["d"][:, par * 512 : (par + 1) * 512],
                            start=True,
                            stop=True,
                        )
                        recip_b = opool.tile([64, 512], f32, tag="recip_b")
                        nc.vector.tensor_copy(out=recip_b, in_=rb_ps)
                        oTx = oT0 if ncc == 0 else oT1
                        nc.vector.tensor_mul(
                            oTx[par * 64 : par * 64 + 64, hp, :],
                            po_t[0:64, :], recip_b,
                        )
                    return st_mul

                return [st_d, st_recip, mk_mul(0, po_e), mk_mul(1, po_o)]

            # ---- the ACT-paced unit stream -------------------------------
            # chunk c = (hp, ncc), ncc-major: heads 2hp, 2hp+1; 512 q-cols.
            chunks = [(hp, ncc) for ncc in range(NCH) for hp in range(KC)]

            # deadline-driven filler (Tile serializes on trace order, so a
            # score matmul emitted before its projection reads garbage):
            #  - todo[ci]: emitted during chunk ci-1, units 0..4
            #  - todo_in[ci]: (unit, closure) emitted at that unit of ci,
            #    just ahead of the score lookahead that consumes it
            todo = {ci: [] for ci in range(1 + KC * NCH)}
            todo_in = {ci: {} for ci in range(KC * NCH)}
            for t in range(1, KC):
                todo[t].append(lambda t=t: k_proj(t, 0))
                todo[t].append(lambda t=t: q_proj(t, 0))
                for mc in range(1, MCH):
                    todo_in[t][2 * (mc - 1)] = lambda t=t, mc=mc: k_proj(t, mc)
            for t in range(KC):
                todo[KC + t].append(lambda t=t: q_proj(t, 1))

            def s_unit(hp, ncc, u):
                """Score matmuls for unit u: T0/T8 row-tiles INTERLEAVED so
                adjacent matmuls sit on opposite PE row-halves — the LDW of
                each pulls ahead of the other's stream and the two halves
                compute concurrently (~2x over a serialized stream)."""
                nsl = slice(ncc * 512, (ncc + 1) * 512)
                psA = ps_s.tile([128, 1024], f32, tag="s")
                psB = ps_s.tile([128, 1024], f32, tag="s")
                tiles = [psA, psB]
                for w in range(2):
                    j = 2 * u + w
                    for par in range(2):   # 0: even head (T0), 1: odd (T8)
                        po = par * 64
                        nc.tensor.matmul(
                            tiles[par][:, w * 512 : (w + 1) * 512],
                            lhsT=kT[po : po + 64, hp, j * 128 : (j + 1) * 128],
                            rhs=qT[po : po + 64, hp, nsl],
                            start=True,
                            stop=True,
                        )
                return tiles

            # discardable matmuls on already-resident weights: keep the PE
            # streaming through DMA-wait windows so the HAM clock gate flips
            # to 2.4GHz early and never re-throttles (output never read)
            def warm(n):
                for _ in range(n // 2):
                    wt = ps_s.tile([128, 1024], f32, tag="s")
                    for w in range(2):
                        nc.tensor.matmul(
                            wt[:, w * 512 : (w + 1) * 512],
                            lhsT=wk[:, w, 0:128],
                            rhs=wk[:, w + 2, 0:512],
                            start=True,
                            stop=True,
                        )

            # prelude: just enough projections for chunk 0's first scores
            k_proj(0, 0)
            warm(6)
            q_proj(0, 0)
            warm(6)

            # flat unit stream: exp(g) | mask pairs | S(g+1) | AV(g-2).
            # AV lags two units so its (exp -> DVE mask) inputs are always
            # ready when the PE, strictly in-order, reaches it.  Chunk
            # retirement (normalize + out-projection) is queued in small
            # stages and dribbled out 2 per unit so it never walls any
            # engine's in-order stream.
            units = [(ci, u) for ci in range(len(chunks)) for u in range(NU)]
            pend = {}                      # (ci, u) -> [psA, psB]
            pts = {}                       # (ci, par, half) -> pt tile
            po_tiles = {}                  # ci -> [po_e, po_o]
            out_due = {3: 0, 4: 1, 5: 2, 6: 3}
            npop = {}                      # ci -> filler drained so far
            retire_q = []
            pend[(0, 0)] = s_unit(0, 0, 0)
            warm(8)
            for j in range(4):
                v_proj(j)

            def emit_av(cj, uj):
                hpj, nccj = chunks[cj]
                if uj == 0:
                    po_a = ps_o.tile([HD + 1, 512], f32, tag="po")
                    po_b = ps_o.tile([HD + 1, 512], f32, tag="po")
                    po_tiles[cj] = [po_a, po_b]
                half = uj // 4
                for w in range(2):         # parity-interleaved: consecutive
                    j = 2 * uj + w         # AVs hit different PSUM banks so
                    for par in range(2):   # fill/drain overlap
                        h = 2 * hpj + par
                        pt_t = pts[(cj, par, half)]
                        nc.tensor.matmul(
                            po_tiles[cj][par],
                            lhsT=vS[:, j, h * (HD + 1) : (h + 1) * (HD + 1)],
                            rhs=pt_t[:, j - 8 * half, :],
                            start=(j == 0),
                            stop=(j == MT - 1),
                        )
                if uj == NU - 1:           # chunk fully accumulated
                    nc_po = po_tiles.pop(cj)
                    retire_q.extend(normalize_pair(nc_po[0], nc_po[1], hpj, nccj))
                    if cj in out_due:
                        retire_q.extend(out_tile_s1(out_due[cj]))

            for g, (ci, u) in enumerate(units):
                hp, ncc = chunks[ci]
                nsl = slice(ncc * 512, (ncc + 1) * 512)
                if retire_q:
                    retire_q.pop(0)()
                warm(2)                # bridge any cross-engine/DMA wait
                # just-in-time K slices for this chunk's later score units
                jit = todo_in.get(ci, {}).pop(u, None)
                if jit is not None:
                    jit()
                tiles = pend.pop((ci, u))
                # exp for both parities (fresh half-chunk pt tile per 4 units)
                half = u // 4
                for par in range(2):
                    if u % 4 == 0:
                        pt_new = ptp.tile([128, NU, 512], bf16, tag="pt")
                        pts[(ci, par, half)] = pt_new
                    pt_sl = pts[(ci, par, half)][:, (u % 4) * 2 : (u % 4) * 2 + 2, :]
                    nc.scalar.activation(pt_sl, tiles[par], Exp)
                # mask applied per unit-pair (2048-wide DVE ops)
                if u % 2 == 1:
                    for par in range(2):
                        lo = (u % 4 - 1) * 2
                        nc.vector.tensor_mul(
                            pts[(ci, par, half)][:, lo : lo + 4, :],
                            pts[(ci, par, half)][:, lo : lo + 4, :],
                            maskT[:, 2 * u - 2 : 2 * u + 2, nsl],
                        )
                # next chunk's start-of-chunk projections (units 0..4)
                nxt = todo.get(ci + 1, [])
                quota = len(nxt) if u >= 4 else (u + 1) * len(nxt) // 5
                npop.setdefault(ci + 1, 0)
                while npop[ci + 1] < quota:
                    nxt[npop[ci + 1]]()
                    npop[ci + 1] += 1
                # lookahead scores for the next unit
                if g + 1 < len(units):
                    nci, nu = units[g + 1]
                    nhp, nncc = chunks[nci]
                    pend[(nci, nu)] = s_unit(nhp, nncc, nu)
                # AV, two units behind
                if g >= 2:
                    emit_av(*units[g - 2])
                # chunk-0 JIT work: rest of K(t=0) + V tiles ahead of AV;
                # V DMAs issued 2 units ahead of their matmuls so the PE's
                # in-order queue never heads-of-line-blocks on a transfer.
                if ci == 0:
                    if u in (0, 2, 4):
                        k_proj(0, u // 2 + 1)
                    if u < 4:
                        v_fetch(2 * u + 8)
                        v_fetch(2 * u + 9)
                    if u < NU - 2:
                        v_proj(2 * u + 4)
                        v_proj(2 * u + 5)
                    warm(2)            # bridge c0's DMA-wait bubbles
            emit_av(*units[-2])
            nq = len(retire_q)
            emit_av(*units[-1])            # pushes the last chunk's stages
            retire_q = retire_q[nq:] + retire_q[:nq]   # last chunk first
            while retire_q:
                retire_q.pop(0)()
                warm(2)                    # keep the clock up into the tail
            tail_stages = [out_tile_s1(nt) for nt in range(4, NT)]
            for st in tail_stages:         # all out-proj matmuls first
                st[0]()
                warm(2)
            for st in tail_stages:
                st[1]()
                st[2]()
                warm(2)
            ln_tail()

    nc.compile()
    return nc


def kernel(**inputs):
    from concourse.bass_utils import run_bass_kernel_spmd

    if "nc" not in _CACHE:
        _CACHE["nc"] = _build()
    nc = _CACHE["nc"]

    query = np.asarray(inputs["query"], dtype=np.float32)
    key = np.asarray(inputs["key"], dtype=np.float32)
    value = np.asarray(inputs["value"], dtype=np.float32)
    mask = np.asarray(inputs["mask"])
    WQ = np.asarray(inputs["WQ"], dtype=np.float32)
    WK = np.asarray(inputs["WK"], dtype=np.float32)
    WV = np.asarray(inputs["WV"], dtype=np.float32)
    WO = np.asarray(inputs["WO"], dtype=np.float32)
    bO = np.asarray(inputs["bO"], dtype=np.float32)
    gamma = np.asarray(inputs["gamma"], dtype=np.float32)
    beta = np.asarray(inputs["beta"], dtype=np.float32)

    scale = np.float32(1.0 / np.sqrt(HD))
    wqT = np.ascontiguousarray(WQ.T * scale).astype(BF16)
    wkT = np.ascontiguousarray(WK.T).astype(BF16)
    wvT = np.ascontiguousarray(WV.T).astype(BF16)
    woT = np.ascontiguousarray(WO.T).astype(BF16)
    gamma_in = gamma.reshape(1, D)
    beta_in = beta.reshape(1, D)
    mask_bin = (mask != 0)

    in_maps = []
    for c in range(NCORES):
        b, n0 = c // 2, (c % 2) * NS
        in_maps.append({
            "xqT": np.ascontiguousarray(query[b, n0 : n0 + NS, :].T).astype(BF16),
            "xkT": np.ascontiguousarray(key[b].T).astype(BF16),
            "xvT": np.ascontiguousarray(value[b].T).astype(BF16),
            "maskT": np.ascontiguousarray(mask_bin[b, n0 : n0 + NS, :].T).astype(BF16),
            "qres": np.ascontiguousarray(query[b, n0 : n0 + NS, :] + bO[None, :]),
            "wqT": wqT, "wkT": wkT, "wvT": wvT, "woT": woT,
            "gamma": gamma_in, "beta": beta_in,
        })

    trace = bool(int(os.environ.get("BASS_KERNEL_TRACE", "0")))
    res = run_bass_kernel_spmd(nc, in_maps, core_ids=list(range(NCORES)), trace=trace)
    _CACHE["last_results"] = res

    out = np.empty((B, N, D), dtype=np.float32)
    for c in range(NCORES):
        b, n0 = c // 2, (c % 2) * NS
        out[b, n0 : n0 + NS, :] = res.results[c]["out"]
    return out


# revision 67
# speedup vs baseline: 1.0302x; 1.0302x over previous
"""MultiHeadGraphAttention TRN2 kernel (v2 — ACT-paced pipeline).

Data-parallel over (batch, query-half): core c handles batch c//2, query rows
(c%2)*1024 .. +1024.  No collectives.  Matmuls in bf16 (fp32 PSUM), softmax
and LayerNorm in fp32.

The softmax exp is the hard engine floor: 16.8M score elements must pass
through ScalarE's ACTIVATE(Exp) at 1 elem/cycle/lane @1.2GHz = ~143us/core.
Everything is organized so ACT runs exp back-to-back and every other engine
hides underneath:

  - scores are computed TRANSPOSED (S^T[m, n]) in [128,1024] PSUM tiles
    (2 key tiles x 512 query cols), exp'd PSUM->SBUF bf16 by ACT, masked by
    DVE (multiply by 0/1 mask AFTER exp), then AV-matmul'd with an appended
    ones-column on V giving the softmax denominator for free.
  - score matmuls for an even/odd head pair are emitted adjacently with
    lhsT/rhs at base_partition 0/64, so they land on PE row-tiles T0/T8
    (64x128 mode) and stream CONCURRENTLY -> scores cost half the cycles.
  - the unit stream runs one unit of score-lookahead ahead of exp; AV and
    the Q/K/V/O projection matmuls trail as PE filler in ACT's slack.
  - LayerNorm: stats on DVE inline, but the Sqrt for all 8 row tiles is
    batched at the very end so ACT never thrashes activation table sets.
"""

import os
import sys

import numpy as np

try:
    import concourse  # noqa: F401
except ImportError:  # harness runs from a bare dir; the repo is a fixed path
    sys.path.insert(0, "/opt/trn_rl_repo")

import ml_dtypes

B, N, M, D, H, HD = 4, 2048, 2048, 512, 8, 64
NS = 1024          # query rows per core
NCORES = 8
LN_EPS = 1e-5
BF16 = ml_dtypes.bfloat16

_CACHE = {}

KC = D // 128      # 4 contraction chunks of 128
NT = NS // 128     # 8 query-row tiles
NCH = NS // 512    # 2 query-column chunks
MT = M // 128      # 16 key-position tiles
MCH = M // 512     # 4 key chunks of 512
NU = MT // 2       # 8 units per chunk (2 key tiles each)


def _build():
    import concourse.bass as bass  # noqa: F401
    import concourse.tile as tile
    from concourse import bacc, mybir
    from concourse.masks import make_identity

    f32 = mybir.dt.float32
    bf16 = mybir.dt.bfloat16
    Exp = mybir.ActivationFunctionType.Exp
    Sqrt = mybir.ActivationFunctionType.Sqrt
    sub = mybir.AluOpType.subtract
    mult = mybir.AluOpType.mult

    nc = bacc.Bacc(None, target_bir_lowering=False, debug=False)

    xqT_d = nc.dram_tensor("xqT", [D, NS], bf16, kind="ExternalInput")
    xkT_d = nc.dram_tensor("xkT", [D, M], bf16, kind="ExternalInput")
    xvT_d = nc.dram_tensor("xvT", [D, M], bf16, kind="ExternalInput")
    maskT_d = nc.dram_tensor("maskT", [M, NS], bf16, kind="ExternalInput")
    qres_d = nc.dram_tensor("qres", [NS, D], f32, kind="ExternalInput")
    wqT_d = nc.dram_tensor("wqT", [D, D], bf16, kind="ExternalInput")
    wkT_d = nc.dram_tensor("wkT", [D, D], bf16, kind="ExternalInput")
    wvT_d = nc.dram_tensor("wvT", [D, D], bf16, kind="ExternalInput")
    woT_d = nc.dram_tensor("woT", [D, D], bf16, kind="ExternalInput")
    gamma_d = nc.dram_tensor("gamma", [1, D], f32, kind="ExternalInput")
    beta_d = nc.dram_tensor("beta", [1, D], f32, kind="ExternalInput")
    out_d = nc.dram_tensor("out", [NS, D], f32, kind="ExternalOutput")

    with tile.TileContext(nc) as tc:
        with (
            tc.tile_pool(name="big", bufs=1) as big,
            tc.tile_pool(name="wpool", bufs=1) as wpool,
            tc.tile_pool(name="ptp", bufs=5) as ptp,
            tc.tile_pool(name="ypool", bufs=2) as ypool,
            tc.tile_pool(name="opool", bufs=2) as opool,
            tc.tile_pool(name="small", bufs=4) as small,
            tc.tile_pool(name="nsc", bufs=1) as nsc,
            tc.tile_pool(name="xvp", bufs=8) as xvp,
            tc.tile_pool(name="ps_s", bufs=2, space="PSUM") as ps_s,
            tc.tile_pool(name="ps_mm", bufs=2, space="PSUM") as ps_mm,
            tc.tile_pool(name="ps_o", bufs=2, space="PSUM") as ps_o,
        ):
            # ---- resident SBUF tensors -----------------------------------
            xqT = big.tile([128, KC, NS], bf16, tag="xqT")
            xkT = big.tile([128, KC, M], bf16, tag="xkT")
            maskT = big.tile([128, MT, NS], bf16, tag="maskT")
            qT = big.tile([128, KC, NS], bf16, tag="qT")
            kT = big.tile([128, KC, M], bf16, tag="kT")
            vS = big.tile([128, MT, H * (HD + 1)], bf16, tag="vS")
            oT0 = big.tile([128, KC, 512], bf16, tag="oT0")
            oT1 = big.tile([128, KC, 512], bf16, tag="oT1")
            ones64 = wpool.tile([1, 64], f32, tag="ones64")
            wq = wpool.tile([128, KC, D], bf16, tag="wq")
            wk = wpool.tile([128, KC, D], bf16, tag="wk")
            wv = wpool.tile([128, KC, D], bf16, tag="wv")
            wo = wpool.tile([128, KC, D], bf16, tag="wo")
            gamma_b = wpool.tile([128, D], f32, tag="gamma_b")
            beta_b = wpool.tile([128, D], f32, tag="beta_b")
            gamma_1 = wpool.tile([1, D], f32, tag="gamma_1")
            beta_1 = wpool.tile([1, D], f32, tag="beta_1")
            eps_t = wpool.tile([128, 1], f32, tag="eps")
            ident = wpool.tile([128, 128], f32, tag="ident")
            # LN staging (persist until the batched tail)
            vars8 = wpool.tile([128, NT], f32, tag="vars8")
            g_all = wpool.tile([128, NT, D], f32, tag="g_all")
            make_identity(nc, ident)

            # ---- input DMAs (critical-path slices first) -----------------
            xq_r = xqT_d[:].rearrange("(c p) n -> p c n", p=128)
            xk_r = xkT_d[:].rearrange("(c p) n -> p c n", p=128)
            mask_r = maskT_d[:].rearrange("(j p) n -> p j n", p=128)
            nc.sync.dma_start(out=wk, in_=wkT_d[:].rearrange("(c p) o -> p c o", p=128))
            nc.sync.dma_start(out=xkT[:, :, 0:512], in_=xk_r[:, :, 0:512])
            nc.sync.dma_start(out=wq, in_=wqT_d[:].rearrange("(c p) o -> p c o", p=128))
            nc.sync.dma_start(out=xqT[:, :, 0:512], in_=xq_r[:, :, 0:512])
            for mc in range(1, MCH):
                nc.sync.dma_start(
                    out=xkT[:, :, mc * 512 : (mc + 1) * 512],
                    in_=xk_r[:, :, mc * 512 : (mc + 1) * 512],
                )
            nc.sync.dma_start(out=xqT[:, :, 512:1024], in_=xq_r[:, :, 512:1024])
            nc.sync.dma_start(out=wv, in_=wvT_d[:].rearrange("(c p) o -> p c o", p=128))
            xvT_r = xvT_d[:].rearrange("(c p) (j m) -> p c j m", p=128, m=128)
            xv_tiles = {}
            for j in range(8):             # prefetch V slices ahead of masks
                xv_t = xvp.tile([128, KC, 128], bf16, tag="xv")
                nc.sync.dma_start(out=xv_t, in_=xvT_r[:, :, j, :])
                xv_tiles[j] = xv_t
            for j in range(MT):
                nc.sync.dma_start(out=maskT[:, j, :], in_=mask_r[:, j, :])
            nc.sync.dma_start(out=wo, in_=woT_d[:].rearrange("(c p) o -> p c o", p=128))
            nc.sync.dma_start(out=gamma_1, in_=gamma_d[:])
            nc.sync.dma_start(out=beta_1, in_=beta_d[:])
            nc.gpsimd.partition_broadcast(gamma_b, gamma_1, channels=128)
            nc.gpsimd.partition_broadcast(beta_b, beta_1, channels=128)
            nc.vector.memset(eps_t, LN_EPS)
            nc.vector.memset(ones64, 1.0)
            nc.vector.memset(
                vS[:].rearrange("p j (h x) -> p j h x", x=HD + 1)[:, :, :, HD : HD + 1],
                1.0,
            )

            # ---- projection emitters (PE filler work) --------------------
            def q_proj(t, ncc):
                ps = ps_mm.tile([128, 512], f32, tag="mm")
                for kc in range(KC):
                    nc.tensor.matmul(
                        ps,
                        lhsT=wq[:, kc, t * 128 : (t + 1) * 128],
                        rhs=xqT[:, kc, ncc * 512 : (ncc + 1) * 512],
                        start=(kc == 0),
                        stop=(kc == KC - 1),
                    )
                nc.vector.tensor_copy(
                    out=qT[:, t, ncc * 512 : (ncc + 1) * 512], in_=ps
                )

            def k_proj(t, mc):
                ps = ps_mm.tile([128, 512], f32, tag="mm")
                for kc in range(KC):
                    nc.tensor.matmul(
                        ps,
                        lhsT=wk[:, kc, t * 128 : (t + 1) * 128],
                        rhs=xkT[:, kc, mc * 512 : (mc + 1) * 512],
                        start=(kc == 0),
                        stop=(kc == KC - 1),
                    )
                nc.vector.tensor_copy(
                    out=kT[:, t, mc * 512 : (mc + 1) * 512], in_=ps
                )

            def v_fetch(j):
                xv_t = xvp.tile([128, KC, 128], bf16, tag="xv")
                nc.sync.dma_start(out=xv_t, in_=xvT_r[:, :, j, :])
                xv_tiles[j] = xv_t

            def v_proj(j):
                xv_t = xv_tiles.pop(j)
                ps = ps_mm.tile([128, 512], f32, tag="mm")
                for kc in range(KC):
                    nc.tensor.matmul(
                        ps,
                        lhsT=xv_t[:, kc, :],
                        rhs=wv[:, kc, :],
                        start=(kc == 0),
                        stop=(kc == KC - 1),
                    )
                # alternate evacuation engine: keep DVE under the ACT pace
                dst = vS[:, j, :].rearrange("p (h x) -> p h x", x=HD + 1)[:, :, 0:HD]
                src = ps[:].rearrange("p (h x) -> p h x", x=HD)
                if j % 2 == 0:
                    nc.vector.tensor_copy(out=dst, in_=src)
                else:
                    nc.scalar.copy(dst, src)

            # ---- out-projection stage 1 (per row tile, inline) -----------
            qres_r = qres_d[:].rearrange("(t p) d -> p t d", p=128)
            out_r = out_d[:].rearrange("(t p) d -> p t d", p=128)

            def out_tile_s1(nt):
                """Returns the out-projection + LN-stats work as stages."""
                box = {}

                def st_mm():
                    oTx = oT0 if nt < 4 else oT1
                    nb = (nt % 4) * 128
                    ps = ps_mm.tile([128, D], f32, tag="mm")
                    for a in range(KC):
                        nc.tensor.matmul(
                            ps,
                            lhsT=oTx[:, a, nb : nb + 128],
                            rhs=wo[:, a, :],
                            start=(a == 0),
                            stop=(a == KC - 1),
                        )
                    x_t = ypool.tile([128, D], f32, tag="qres")
                    nc.sync.dma_start(out=x_t, in_=qres_r[:, nt, :])
                    box["ps"], box["x"] = ps, x_t

                def st_stats():
                    ps, x_t = box["ps"], box["x"]
                    nc.vector.tensor_add(x_t, ps, x_t)
                    stats = small.tile([128, 6], f32, tag="stats")
                    nc.vector.bn_stats(out=stats, in_=x_t)
                    mv = small.tile([128, 2], f32, tag="mv")
                    nc.vector.bn_aggr(out=mv, in_=stats)
                    nc.vector.tensor_copy(out=vars8[:, nt : nt + 1], in_=mv[:, 1:2])
                    box["mv"] = mv

                def st_gam():
                    nc.vector.tensor_scalar(
                        out=g_all[:, nt, :], in0=box["x"], scalar1=box["mv"][:, 0:1],
                        scalar2=None, op0=sub,
                    )
                    nc.gpsimd.tensor_mul(g_all[:, nt, :], g_all[:, nt, :], gamma_b)

                return [st_mm, st_stats, st_gam]

            def ln_tail():
                rstd = wpool.tile([128, NT], f32, tag="rstd8")
                nc.scalar.activation(rstd, vars8, Sqrt, bias=eps_t)
                nc.vector.reciprocal(rstd, rstd)
                for nt in range(NT):
                    y_t = g_all[:, nt, :]  # in place: g dead after this
                    nc.vector.tensor_scalar(
                        out=y_t,
                        in0=y_t,
                        scalar1=rstd[:, nt : nt + 1],
                        scalar2=None,
                        op0=mult,
                    )
                    if nt % 2 == 0:        # split the serial tail chain
                        nc.gpsimd.tensor_add(y_t, y_t, beta_b)
                    else:
                        nc.vector.tensor_add(y_t, y_t, beta_b)
                    nc.sync.dma_start(out=out_r[:, nt, :], in_=y_t)

            # ---- softmax denominator reciprocal (PE transpose trick) -----
            # both heads of a chunk batched: d rows staged to SBUF, PE
            # transposes spread them over partitions so DVE reciprocal runs
            # lane-parallel, transposes back, broadcast, multiply.
            def normalize_pair(po_e, po_o, hp, ncc):
                """Returns the softmax-denominator normalize work as stages."""
                nsl = slice(ncc * 512, (ncc + 1) * 512)
                box = {}

                def st_d():
                    d_sb = nsc.tile([1, 1024], f32, tag="d_sb")
                    nc.vector.tensor_copy(out=d_sb[:, 0:512], in_=po_e[64:65, :])
                    nc.vector.tensor_copy(out=d_sb[:, 512:1024], in_=po_o[64:65, :])
                    box["d"] = d_sb

                def st_recip():
                    d_sb = box["d"]
                    scr = ps_mm.tile([128, 512], f32, tag="mm")
                    dT = scr[:, 0:8]
                    for c in range(2 * KC):
                        nc.tensor.transpose(
                            dT[:, c : c + 1],
                            d_sb[:, c * 128 : (c + 1) * 128],
                            ident[0:1, 0:1],
                        )
                    rT = small.tile([128, 8], f32, tag="rT")
                    nc.vector.reciprocal(rT, dT)
                    for half in range(2):
                        rr = scr[0:1, 0:512]
                        for c in range(KC):
                            nc.tensor.transpose(
                                rr[:, c * 128 : (c + 1) * 128],
                                rT[:, 4 * half + c : 4 * half + c + 1],
                                ident,
                            )
                        nc.vector.tensor_copy(
                            out=d_sb[:, half * 512 : (half + 1) * 512], in_=rr
                        )

                def mk_mul(par, po_t):
                    def st_mul():
                        # broadcast 1/d over 64 partitions via a PE rank-1
                        # outer product (no gpsimd in this chain)
                        rb_ps = ps_mm.tile([64, 512], f32, tag="mm")
                        nc.tensor.matmul(
                            rb_ps,
                            lhsT=ones64,
                            rhs=box["d"][:, par * 512 : (par + 1) * 512],
                            start=True,
                            stop=True,
                        )
                        recip_b = opool.tile([64, 512], f32, tag="recip_b")
                        nc.vector.tensor_copy(out=recip_b, in_=rb_ps)
                        oTx = oT0 if ncc == 0 else oT1
                        nc.vector.tensor_mul(
                            oTx[par * 64 : par * 64 + 64, hp, :],
                            po_t[0:64, :], recip_b,
                        )
                    return st_mul

                return [st_d, st_recip, mk_mul(0, po_e), mk_mul(1, po_o)]

            # ---- the ACT-paced unit stream -------------------------------
            # chunk c = (hp, ncc), ncc-major: heads 2hp, 2hp+1; 512 q-cols.
            chunks = [(hp, ncc) for ncc in range(NCH) for hp in range(KC)]

            # deadline-driven filler (Tile serializes on trace order, so a
            # score matmul emitted before its projection reads garbage):
            #  - todo[ci]: emitted during chunk ci-1, units 0..4
            #  - todo_in[ci]: (unit, closure) emitted at that unit of ci,
            #    just ahead of the score lookahead that consumes it
            todo = {ci: [] for ci in range(1 + KC * NCH)}
            todo_in = {ci: {} for ci in range(KC * NCH)}
            for t in range(1, KC):
                todo[t].append(lambda t=t: k_proj(t, 0))
                todo[t].append(lambda t=t: q_proj(t, 0))
                for mc in range(1, MCH):
                    todo_in[t][2 * (mc - 1)] = lambda t=t, mc=mc: k_proj(t, mc)
            for t in range(KC):
                todo[KC + t].append(lambda t=t: q_proj(t, 1))

            def s_unit(hp, ncc, u):
                """Score matmuls for unit u: T0/T8 row-tiles INTERLEAVED so
                adjacent matmuls sit on opposite PE row-halves — the LDW of
                each pulls ahead of the other's stream and the two halves
                compute concurrently (~2x over a serialized stream)."""
                nsl = slice(ncc * 512, (ncc + 1) * 512)
                psA = ps_s.tile([128, 1024], f32, tag="s")
                psB = ps_s.tile([128, 1024], f32, tag="s")
                tiles = [psA, psB]
                for w in range(2):
                    j = 2 * u + w
                    for par in range(2):   # 0: even head (T0), 1: odd (T8)
                        po = par * 64
                        nc.tensor.matmul(
                            tiles[par][:, w * 512 : (w + 1) * 512],
                            lhsT=kT[po : po + 64, hp, j * 128 : (j + 1) * 128],
                            rhs=qT[po : po + 64, hp, nsl],
                            start=True,
                            stop=True,
                        )
                return tiles

            # discardable matmuls on already-resident weights: keep the PE
            # streaming through DMA-wait windows so the HAM clock gate flips
            # to 2.4GHz early and never re-throttles (output never read)
            def warm(n):
                for _ in range(n // 2):
                    wt = ps_s.tile([128, 1024], f32, tag="s")
                    for w in range(2):
                        nc.tensor.matmul(
                            wt[:, w * 512 : (w + 1) * 512],
                            lhsT=wk[:, w, 0:128],
                            rhs=wk[:, w + 2, 0:512],
                            start=True,
                            stop=True,
                        )

            # prelude: just enough projections for chunk 0's first scores
            k_proj(0, 0)
            warm(6)
            q_proj(0, 0)
            warm(6)

            # flat unit stream: exp(g) | mask pairs | S(g+1) | AV(g-2).
            # AV lags two units so its (exp -> DVE mask) inputs are always
            # ready when the PE, strictly in-order, reaches it.  Chunk
            # retirement (normalize + out-projection) is queued in small
            # stages and dribbled out 2 per unit so it never walls any
            # engine's in-order stream.
            units = [(ci, u) for ci in range(len(chunks)) for u in range(NU)]
            pend = {}                      # (ci, u) -> [psA, psB]
            pts = {}                       # (ci, par, half) -> pt tile
            po_tiles = {}                  # ci -> [po_e, po_o]
            out_due = {3: 0, 4: 1, 5: 2, 6: 3}
            npop = {}                      # ci -> filler drained so far
            retire_q = []
            pend[(0, 0)] = s_unit(0, 0, 0)
            warm(8)
            for j in range(4):
                v_proj(j)

            def emit_av(cj, uj):
                hpj, nccj = chunks[cj]
                if uj == 0:
                    po_a = ps_o.tile([HD + 1, 512], f32, tag="po")
                    po_b = ps_o.tile([HD + 1, 512], f32, tag="po")
                    po_tiles[cj] = [po_a, po_b]
                half = uj // 4
                for w in range(2):         # parity-interleaved: consecutive
                    j = 2 * uj + w         # AVs hit different PSUM banks so
                    for par in range(2):   # fill/drain overlap
                        h = 2 * hpj + par
                        pt_t = pts[(cj, par, half)]
                        nc.tensor.matmul(
                            po_tiles[cj][par],
                            lhsT=vS[:, j, h * (HD + 1) : (h + 1) * (HD + 1)],
                            rhs=pt_t[:, j - 8 * half, :],
                            start=(j == 0),
                            stop=(j == MT - 1),
                        )
                if uj == NU - 1:           # chunk fully accumulated
                    nc_po = po_tiles.pop(cj)
                    retire_q.extend(normalize_pair(nc_po[0], nc_po[1], hpj, nccj))
                    if cj in out_due:
                        retire_q.extend(out_tile_s1(out_due[cj]))

            for g, (ci, u) in enumerate(units):
                hp, ncc = chunks[ci]
                nsl = slice(ncc * 512, (ncc + 1) * 512)
                if retire_q:
                    retire_q.pop(0)()
                    warm(2)            # bridge any cross-engine chain wait
                # just-in-time K slices for this chunk's later score units
                jit = todo_in.get(ci, {}).pop(u, None)
                if jit is not None:
                    jit()
                tiles = pend.pop((ci, u))
                # exp for both parities (fresh half-chunk pt tile per 4 units)
                half = u // 4
                for par in range(2):
                    if u % 4 == 0:
                        pt_new = ptp.tile([128, NU, 512], bf16, tag="pt")
                        pts[(ci, par, half)] = pt_new
                    pt_sl = pts[(ci, par, half)][:, (u % 4) * 2 : (u % 4) * 2 + 2, :]
                    nc.scalar.activation(pt_sl, tiles[par], Exp)
                # mask applied per unit-pair (2048-wide DVE ops)
                if u % 2 == 1:
                    for par in range(2):
                        lo = (u % 4 - 1) * 2
                        nc.vector.tensor_mul(
                            pts[(ci, par, half)][:, lo : lo + 4, :],
                            pts[(ci, par, half)][:, lo : lo + 4, :],
                            maskT[:, 2 * u - 2 : 2 * u + 2, nsl],
                        )
                # next chunk's start-of-chunk projections (units 0..4)
                nxt = todo.get(ci + 1, [])
                quota = len(nxt) if u >= 4 else (u + 1) * len(nxt) // 5
                npop.setdefault(ci + 1, 0)
                while npop[ci + 1] < quota:
                    nxt[npop[ci + 1]]()
                    npop[ci + 1] += 1
                # lookahead scores for the next unit
                if g + 1 < len(units):
                    nci, nu = units[g + 1]
                    nhp, nncc = chunks[nci]
                    pend[(nci, nu)] = s_unit(nhp, nncc, nu)
                # AV, two units behind
                if g >= 2:
                    emit_av(*units[g - 2])
                # chunk-0 JIT work: rest of K(t=0) + V tiles ahead of AV;
                # V DMAs issued 2 units ahead of their matmuls so the PE's
                # in-order queue never heads-of-line-blocks on a transfer.
                if ci == 0:
                    if u in (0, 2, 4):
                        k_proj(0, u // 2 + 1)
                    if u < 4:
                        v_fetch(2 * u + 8)
                        v_fetch(2 * u + 9)
                    if u < NU - 2:
                        v_proj(2 * u + 4)
                        v_proj(2 * u + 5)
                    warm(2)            # bridge c0's DMA-wait bubbles
            emit_av(*units[-2])
            nq = len(retire_q)
            emit_av(*units[-1])            # pushes the last chunk's stages
            retire_q = retire_q[nq:] + retire_q[:nq]   # last chunk first
            while retire_q:
                retire_q.pop(0)()
                warm(2)                    # keep the clock up into the tail
            tail_stages = [out_tile_s1(nt) for nt in range(4, NT)]
            for st in tail_stages:         # all out-proj matmuls first
                st[0]()
                warm(2)
            for st in tail_stages:
                st[1]()
                st[2]()
                warm(2)
            ln_tail()

    nc.compile()
    return nc


def kernel(**inputs):
    from concourse.bass_utils import run_bass_kernel_spmd

    if "nc" not in _CACHE:
        _CACHE["nc"] = _build()
    nc = _CACHE["nc"]

    query = np.asarray(inputs["query"], dtype=np.float32)
    key = np.asarray(inputs["key"], dtype=np.float32)
    value = np.asarray(inputs["value"], dtype=np.float32)
    mask = np.asarray(inputs["mask"])
    WQ = np.asarray(inputs["WQ"], dtype=np.float32)
    WK = np.asarray(inputs["WK"], dtype=np.float32)
    WV = np.asarray(inputs["WV"], dtype=np.float32)
    WO = np.asarray(inputs["WO"], dtype=np.float32)
    bO = np.asarray(inputs["bO"], dtype=np.float32)
    gamma = np.asarray(inputs["gamma"], dtype=np.float32)
    beta = np.asarray(inputs["beta"], dtype=np.float32)

    scale = np.float32(1.0 / np.sqrt(HD))
    wqT = np.ascontiguousarray(WQ.T * scale).astype(BF16)
    wkT = np.ascontiguousarray(WK.T).astype(BF16)
    wvT = np.ascontiguousarray(WV.T).astype(BF16)
    woT = np.ascontiguousarray(WO.T).astype(BF16)
    gamma_in = gamma.reshape(1, D)
    beta_in = beta.reshape(1, D)
    mask_bin = (mask != 0)

    in_maps = []
    for c in range(NCORES):
        b, n0 = c // 2, (c % 2) * NS
        in_maps.append({
            "xqT": np.ascontiguousarray(query[b, n0 : n0 + NS, :].T).astype(BF16),
            "xkT": np.ascontiguousarray(key[b].T).astype(BF16),
            "xvT": np.ascontiguousarray(value[b].T).astype(BF16),
            "maskT": np.ascontiguousarray(mask_bin[b, n0 : n0 + NS, :].T).astype(BF16),
            "qres": np.ascontiguousarray(query[b, n0 : n0 + NS, :] + bO[None, :]),
            "wqT": wqT, "wkT": wkT, "wvT": wvT, "woT": woT,
            "gamma": gamma_in, "beta": beta_in,
        })

    trace = bool(int(os.environ.get("BASS_KERNEL_TRACE", "0")))
    res = run_bass_kernel_spmd(nc, in_maps, core_ids=list(range(NCORES)), trace=trace)
    _CACHE["last_results"] = res

    out = np.empty((B, N, D), dtype=np.float32)
    for c in range(NCORES):
        b, n0 = c // 2, (c % 2) * NS
        out[b, n0 : n0 + NS, :] = res.results[c]["out"]
    return out
